# revision 54
# baseline (speedup 1.0000x reference)
"""Distributed Bass kernel for nn_Attention_20993800143414.

Reference computation (B=2, S=2048, C=256, H=8, D=32):
    q = (q_x @ Wq.T) * D**-0.5 ; k = kv_x @ Wk.T ; v = kv_x @ Wv.T
    scores = einsum("bqhd,bkhd->bhqk", q, k) + attn_bias
    w = softmax(scores, -1)
    o = einsum("bhqk,bkhd->bqhd", w, v).reshape(b, s, C) @ Wout.T + b_out
    out = o * sigmoid(q_x @ Wg.T + b_g + gating_bias)

Sharding: 16 (b,h) pairs -> 8 cores, 2 heads of one batch per core.
Each core computes its 2 heads' attention + the partial out-projection
(+ b_out/4 rank-1 term) multiplied by the gating; the host sums the
4 partials per batch (pure unshard-reduce for head parallelism).

Layout: scores are built TRANSPOSED, sT[k,q] (k on partitions):
  - host ships exp(biasT) bf16; after ACT exp of the qk scores the bias
    becomes a DVE bf16 multiply (exp(s+b) = exp(s)*exp(b)).
  - PV is col-group packed: 4 concurrent M=32 matmuls (tile_position
    (0,32n)) write a FOLDED oT psum [128,512] (rows 32n = q-chunk n),
    one PSUM bank, ~1 stream-time per k-tile.
  - softmax denominators: 4 concurrent M=1 matmuls with a ones vector
    into den_ps[97,512] rows 32n.
  - out-projection consumes folded oT directly as lhsT; 1/denominator
    is applied afterwards as a per-partition scalar.
  - the PV/den wave lags QK by one k-tile (software pipeline) so the
    PE stream stays dense.
"""

import sys

for _p in ("/opt/trn_rl_repo",):
    if _p not in sys.path:
        sys.path.insert(0, _p)

import numpy as np
import ml_dtypes
from contextlib import ExitStack

import concourse.bass as bass
import concourse.bacc as bacc
import concourse.mybir as mybir
import concourse.tile as tile
from concourse.bass import ds, ts
from concourse.bass_utils import run_bass_kernel_spmd
from concourse.masks import make_identity

B, S, C, H, D = 2, 2048, 256, 8, 32
NCORES = 8
HPC = (B * H) // NCORES  # heads per core = 2
HD = HPC * D  # 64
QT = S // 128  # 16 q (and k) tiles
NCH = S // 512  # 4 free-dim chunks of 512
BF16 = mybir.dt.bfloat16
F32 = mybir.dt.float32
EXPF = mybir.ActivationFunctionType.Exp
SIGF = mybir.ActivationFunctionType.Sigmoid
COPYF = mybir.ActivationFunctionType.Copy

_NC_CACHE = {}


def build_nc():
    nc = bacc.Bacc("TRN2", target_bir_lowering=False, debug=False, num_devices=NCORES)

    xq = nc.dram_tensor("xq", [C, S], BF16, kind="ExternalInput").ap()
    xkv = nc.dram_tensor("xkv", [C, S], BF16, kind="ExternalInput").ap()
    biasT = nc.dram_tensor("biasT", [HPC, S, S], BF16, kind="ExternalInput").ap()
    wq = nc.dram_tensor("wq", [C, HD], BF16, kind="ExternalInput").ap()
    wk = nc.dram_tensor("wk", [C, HD], BF16, kind="ExternalInput").ap()
    wv = nc.dram_tensor("wv", [C, HD], BF16, kind="ExternalInput").ap()
    wo = nc.dram_tensor("wo", [HD, C], BF16, kind="ExternalInput").ap()
    wg = nc.dram_tensor("wg", [C, C], BF16, kind="ExternalInput").ap()
    browg = nc.dram_tensor("browg", [1, C], BF16, kind="ExternalInput").ap()
    browo = nc.dram_tensor("browo", [1, C], BF16, kind="ExternalInput").ap()
    out = nc.dram_tensor("out", [S, C], F32, kind="ExternalOutput").ap()

    with tile.TileContext(nc) as tc, ExitStack() as ctx:
        consts = ctx.enter_context(tc.tile_pool(name="consts", bufs=1))
        sb = ctx.enter_context(tc.tile_pool(name="sb", bufs=1))
        bias_pool = ctx.enter_context(tc.tile_pool(name="bias", bufs=2))
        exp_pool = ctx.enter_context(tc.tile_pool(name="exp", bufs=3))
        res_pool = ctx.enter_context(tc.tile_pool(name="res", bufs=6))
        ps_s = ctx.enter_context(tc.tile_pool(name="ps_s", bufs=3, space="PSUM"))
        ps_o = ctx.enter_context(tc.tile_pool(name="ps_o", bufs=1, space="PSUM"))

        # ---- constants ----
        id97 = consts.tile([97, 97], F32)
        make_identity(nc, id97[:])
        ones_r = consts.tile([1, 128], BF16)
        nc.vector.memset(ones_r[:], 1.0)
        ones_c = consts.tile([128, 1], BF16)
        nc.vector.memset(ones_c[:], 1.0)

        # ---- DMAs ordered by first consumer: gating needs wg + xq
        #      chunk 0; projections need wq/wk + the rest of x ----
        def load_w2(name, dram, m):
            t = consts.tile([128, 2 * m], BF16, tag=name, name=name + "_sb")
            nc.sync.dma_start(
                t[:].rearrange("p (j m) -> p j m", j=2),
                dram.rearrange("(j p) m -> p j m", p=128),
            )
            return t

        xq_sb = sb.tile([128, 2 * S], BF16)
        xkv_sb = sb.tile([128, 2 * S], BF16)

        def load_x(t_, dram, n):
            dst = t_[:].rearrange("p (j s) -> p j s", j=2)
            src = dram.rearrange("(j p) s -> p j s", p=128)
            nc.sync.dma_start(
                dst[:, :, ds(n * 512, 512)], src[:, :, ds(n * 512, 512)]
            )

        wg_sb = load_w2("wg", wg, C)
        browg_sb = consts.tile([1, C], BF16)
        nc.sync.dma_start(browg_sb[:], browg)
        browo_sb = consts.tile([1, C], BF16)
        nc.sync.dma_start(browo_sb[:], browo)
        load_x(xq_sb, xq, 0)
        wq_sb = load_w2("wq", wq, HD)
        wk_sb = load_w2("wk", wk, HD)
        wv_sb = load_w2("wv", wv, HD)
        load_x(xkv_sb, xkv, 0)
        for n in range(1, NCH):
            load_x(xq_sb, xq, n)
            load_x(xkv_sb, xkv, n)
        wo_sb = consts.tile([HD, C], BF16)
        nc.sync.dma_start(wo_sb[:], wo)

        # ---- projections: qT, kT [HD, S] bf16 (d on partitions),
        #      col-packed: qT in rows 0-63, kT in rows 64-127 ----
        qT = sb.tile([HD, S], BF16)
        kT = sb.tile([HD, S], BF16)
        for n in range(NCH):
            ps = ps_s.tile([128, 512], F32, tag="ps", name="ps_proj")
            for j in range(2):
                nc.tensor.matmul(
                    ps[ds(0, HD), :],
                    wq_sb[:, ds(j * HD, HD)],
                    xq_sb[:, ds(j * S + n * 512, 512)],
                    start=(j == 0), stop=(j == 1),
                    tile_position=(0, 0),
                )
                nc.tensor.matmul(
                    ps[ds(HD, HD), :],
                    wk_sb[:, ds(j * HD, HD)],
                    xkv_sb[:, ds(j * S + n * 512, 512)],
                    start=(j == 0), stop=(j == 1),
                    tile_position=(0, 64),
                )
            nc.vector.tensor_copy(qT[:, ds(n * 512, 512)], ps[ds(0, HD), :])
            nc.vector.tensor_copy(kT[:, ds(n * 512, 512)], ps[ds(HD, HD), :])

        # ---- gating: g = sigmoid(xq.T @ WgT + brow_g); first half in the
        #      prologue, second half in the tail (PE is idle there) ----
        g_all = sb.tile([128, QT * C], F32)

        def emit_gate(t):
            psg = ps_s.tile([128, C], F32, tag="ps", name="psg")
            for j in range(2):
                nc.tensor.matmul(
                    psg[:],
                    xq_sb[:, ds(j * S + t * 128, 128)],
                    wg_sb[:, ds(j * C, C)],
                    start=(j == 0), stop=False,
                )
            nc.tensor.matmul(psg[:], ones_r[:], browg_sb[:], start=False, stop=True)
            nc.scalar.activation(g_all[:, ds(t * C, C)], psg[:], SIGF)

        for t in range(QT // 2):
            emit_gate(t)

        # ---- b_out/4 broadcast to [128, C] ----
        bout_bc = consts.tile([128, C], F32)
        ps_b = ps_s.tile([128, C], F32, tag="ps")
        nc.tensor.matmul(ps_b[:], ones_r[:], browo_sb[:], start=True, stop=True)
        nc.vector.tensor_copy(bout_bc[:], ps_b[:])

        # ---- V natural layout: v_sb[:, kt*HD ...]; tiles are emitted
        #      lazily (2-kt lookahead) inside the score loop ----
        v_sb = sb.tile([128, QT * HPC * D], BF16)

        def emit_v(t):
            ps = ps_s.tile([128, HD], F32, tag="ps", name="ps_v")
            for j in range(2):
                nc.tensor.matmul(
                    ps[:],
                    xkv_sb[:, ds(j * S + t * 128, 128)],
                    wv_sb[:, ds(j * HD, HD)],
                    start=(j == 0),
                    stop=(j == 1),
                )
            nc.vector.tensor_copy(v_sb[:, ds(t * HD, HD)], ps[:])

        emit_v(0)
        emit_v(1)

        # ---- attention per head, software-pipelined (PV lags QK by 1 kt);
        #      head-0's epilogue half is spread through head-1's loop ----
        # folded layouts: oT_fold rows 32n = oT[0:32, 512n:512n+512]
        oT_sb = sb.tile([128, HPC * 512], BF16)
        oT_unf = sb.tile([HD, S], BF16)  # head i rows 32i, natural q columns
        den_sb = sb.tile([97, HPC * 512], F32)
        r97 = sb.tile([128, HPC * NCH * 97], F32)
        res0_all = sb.tile([128, QT * C], F32)
        oT_tiles = {}
        den_tiles = {}

        def emit_den_chain(i):
            """den psum -> den_sb -> transposed reciprocals in r97."""
            nc.vector.tensor_copy(den_sb[:, ds(i * 512, 512)], den_tiles[i][:])
            for c in range(NCH):
                trp = ps_s.tile([128, 97], F32, tag="ps", name="trp")
                nc.tensor.transpose(
                    trp[:], den_sb[:, ds(i * 512 + c * 128, 128)], id97[:]
                )
                nc.vector.reciprocal(
                    r97[:, ds((i * NCH + c) * 97, 97)][:, 0:97:32],
                    trp[:, 0:97:32],
                )

        def emit_oproj(i, t):
            """out-projection of head i for q-tile t, scaled by 1/den."""
            n, c = t // 4, t % 4
            ps = ps_s.tile([128, C], F32, tag="ps", name="ps_op")
            nc.tensor.matmul(
                ps[:],
                oT_unf[ds(i * D, D), ds(t * 128, 128)],
                wo_sb[ds(i * D, D), :],
                start=True, stop=True,
            )
            r_ap = r97[:, ds((i * NCH + c) * 97 + 32 * n, 1)]
            if i == 0:
                res0 = res0_all[:, ds(t * C, C)]
                nc.scalar.activation(res0, ps[:], COPYF, scale=r_ap)
                nc.vector.tensor_add(res0, res0, bout_bc[:])
            else:
                if t >= QT // 2:
                    emit_gate(t)
                tmp = res_pool.tile([128, C], F32, tag="tmp", name="tmp")
                nc.scalar.activation(tmp[:], ps[:], COPYF, scale=r_ap)
                res = res_pool.tile([128, C], F32, tag="res", name="res")
                nc.vector.tensor_add(res[:], res0_all[:, ds(t * C, C)], tmp[:])
                nc.vector.tensor_mul(res[:], res[:], g_all[:, ds(t * C, C)])
                nc.sync.dma_start(out[ds(t * 128, 128), :], res[:])

        prev = None
        for j in range(HPC * QT + 1):
            cur = None
            if j < HPC * QT:
                i, kt = j // QT, j % QT
                if kt == 0:
                    oT_tiles[i] = ps_o.tile(
                        [128, 512], F32, tag="ps_oT", name="oT_ps"
                    )
                    den_tiles[i] = ps_o.tile(
                        [97, 512], F32, tag="ps_den", name="den_ps"
                    )
                bt = bias_pool.tile([128, S], BF16)
                nc.sync.dma_start(bt[:], biasT[i, ds(kt * 128, 128), :])
                et = exp_pool.tile([128, S], BF16)
                pss = []
                for n in range(2):
                    ps = ps_s.tile([128, 1024], F32, tag="ps")
                    for m in range(2):
                        nc.tensor.matmul(
                            ps[:, ds(m * 512, 512)],
                            kT[ds(i * D, D), ds(kt * 128, 128)],
                            qT[ds(i * D, D), ds(n * 1024 + m * 512, 512)],
                            start=True, stop=True,
                        )
                    pss.append(ps)
                for n in range(2):
                    nc.scalar.activation(et[:, ds(n * 1024, 1024)], pss[n][:], EXPF)
                    nc.vector.tensor_mul(
                        et[:, ds(n * 1024, 1024)],
                        et[:, ds(n * 1024, 1024)],
                        bt[:, ds(n * 1024, 1024)],
                    )
                if i == 0 and kt + 2 < QT:
                    emit_v(kt + 2)
                cur = (i, kt, et)
            if prev is not None:
                pi, pkt, pet = prev
                st, sp = pkt == 0, pkt == QT - 1
                for n in range(NCH):
                    nc.tensor.matmul(
                        oT_tiles[pi][ds(32 * n, 32), :],
                        v_sb[:, ds((pkt * HPC + pi) * D, D)],
                        pet[:, ds(n * 512, 512)],
                        start=st, stop=sp,
                        tile_position=(0, 32 * n),
                    )
                for n in range(NCH):
                    nc.tensor.matmul(
                        den_tiles[pi][ds(32 * n, 1), :],
                        ones_c[:],
                        pet[:, ds(n * 512, 512)],
                        start=st, stop=sp,
                        tile_position=(0, 32 * n),
                    )
                if sp:
                    nc.vector.tensor_copy(
                        oT_sb[:, ds(pi * 512, 512)], oT_tiles[pi][:]
                    )
                    # unfold: oT_unf[32i+p, 512n+j] = oT_sb[32n+p, 512i+j]
                    for n in range(NCH):
                        nc.scalar.dma_start(
                            oT_unf[ds(pi * D, D), ds(n * 512, 512)],
                            oT_sb[ds(32 * n, 32), ds(pi * 512, 512)],
                        )
            # spread head-0 epilogue through head-1's score loop
            if j == QT + 1:
                emit_den_chain(0)
            elif QT + 2 <= j <= QT + 1 + QT // 2:
                base = 2 * (j - QT - 2)
                emit_oproj(0, base)
                emit_oproj(0, base + 1)
            prev = cur

        # ---- tail: head-1 epilogue ----
        emit_den_chain(1)
        for t in range(QT):
            emit_oproj(1, t)

    nc.compile()
    return nc


def _shard_inputs(q_x, kv_x, attn_bias, Wq, Wk, Wv, Wout, b_out, Wg, b_g, gating_bias):
    bf = ml_dtypes.bfloat16
    in_maps = []
    scale = np.float32(D) ** np.float32(-0.5)
    for core in range(NCORES):
        b, hp = core // 4, core % 4
        hsl = slice(hp * HD, (hp + 1) * HD)
        in_maps.append(
            {
                "xq": np.ascontiguousarray(q_x[b].T).astype(bf),
                "xkv": np.ascontiguousarray(kv_x[b].T).astype(bf),
                "biasT": np.exp(
                    np.ascontiguousarray(
                        attn_bias[b, 2 * hp : 2 * hp + 2].transpose(0, 2, 1)
                    )
                ).astype(bf),
                "wq": np.ascontiguousarray((Wq[hsl] * scale).T).astype(bf),
                "wk": np.ascontiguousarray(Wk[hsl].T).astype(bf),
                "wv": np.ascontiguousarray(Wv[hsl].T).astype(bf),
                "wo": np.ascontiguousarray(Wout[:, hsl].T).astype(bf),
                "wg": np.ascontiguousarray(Wg.T).astype(bf),
                "browg": (b_g + gating_bias).reshape(1, C).astype(bf),
                "browo": (b_out / 4.0).reshape(1, C).astype(bf),
            }
        )
    return in_maps


def run(inputs, trace=False, **kw):
    if "nc" not in _NC_CACHE:
        _NC_CACHE["nc"] = build_nc()
    nc = _NC_CACHE["nc"]
    inputs = {k: np.asarray(v, dtype=np.float32) for k, v in inputs.items()}
    in_maps = _shard_inputs(**inputs)
    r = run_bass_kernel_spmd(nc, in_maps, core_ids=list(range(NCORES)), trace=trace, **kw)
    outs = np.stack([np.asarray(m["out"], np.float32) for m in r.results])
    full = outs.reshape(B, 4, S, C).sum(axis=1)
    return full, r


def kernel(**inputs) -> np.ndarray:
    full, _ = run(inputs, trace=False)
    return full


if __name__ == "__main__":
    print("building...")
    build_nc()
    print("ok")


# revision 55
# speedup vs baseline: 1.0219x; 1.0219x over previous
"""Distributed Bass kernel for nn_Attention_20993800143414.

Reference computation (B=2, S=2048, C=256, H=8, D=32):
    q = (q_x @ Wq.T) * D**-0.5 ; k = kv_x @ Wk.T ; v = kv_x @ Wv.T
    scores = einsum("bqhd,bkhd->bhqk", q, k) + attn_bias
    w = softmax(scores, -1)
    o = einsum("bhqk,bkhd->bqhd", w, v).reshape(b, s, C) @ Wout.T + b_out
    out = o * sigmoid(q_x @ Wg.T + b_g + gating_bias)

Sharding: 16 (b,h) pairs -> 8 cores, 2 heads of one batch per core.
Each core computes its 2 heads' attention + the partial out-projection
(+ b_out/4 rank-1 term) multiplied by the gating; the host sums the
4 partials per batch (pure unshard-reduce for head parallelism).

Layout: scores are built TRANSPOSED, sT[k,q] (k on partitions):
  - host ships exp(biasT) bf16; after ACT exp of the qk scores the bias
    becomes a DVE bf16 multiply (exp(s+b) = exp(s)*exp(b)).
  - PV is col-group packed: 4 concurrent M=32 matmuls (tile_position
    (0,32n)) write a FOLDED oT psum [128,512] (rows 32n = q-chunk n),
    one PSUM bank, ~1 stream-time per k-tile.
  - softmax denominators: 4 concurrent M=1 matmuls with a ones vector
    into den_ps[97,512] rows 32n.
  - out-projection consumes folded oT directly as lhsT; 1/denominator
    is applied afterwards as a per-partition scalar.
  - the PV/den wave lags QK by one k-tile (software pipeline) so the
    PE stream stays dense.
"""

import sys

for _p in ("/opt/trn_rl_repo",):
    if _p not in sys.path:
        sys.path.insert(0, _p)

import numpy as np
import ml_dtypes
from contextlib import ExitStack

import concourse.bass as bass
import concourse.bacc as bacc
import concourse.mybir as mybir
import concourse.tile as tile
from concourse.bass import ds
from concourse.bass_utils import run_bass_kernel_spmd
from concourse.masks import make_identity

B, S, C, H, D = 2, 2048, 256, 8, 32
NCORES = 8
HPC = (B * H) // NCORES  # heads per core = 2
HD = HPC * D  # 64
QT = S // 128  # 16 q (and k) tiles
NCH = S // 512  # 4 free-dim chunks of 512
BF16 = mybir.dt.bfloat16
F32 = mybir.dt.float32
EXPF = mybir.ActivationFunctionType.Exp
SIGF = mybir.ActivationFunctionType.Sigmoid
COPYF = mybir.ActivationFunctionType.Copy

_NC_CACHE = {}


def build_nc():
    nc = bacc.Bacc("TRN2", target_bir_lowering=False, debug=False, num_devices=NCORES)

    xq = nc.dram_tensor("xq", [C, S], BF16, kind="ExternalInput").ap()
    xkv = nc.dram_tensor("xkv", [C, S], BF16, kind="ExternalInput").ap()
    biasT = nc.dram_tensor("biasT", [HPC, S, S], BF16, kind="ExternalInput").ap()
    wq = nc.dram_tensor("wq", [C, HD], BF16, kind="ExternalInput").ap()
    wk = nc.dram_tensor("wk", [C, HD], BF16, kind="ExternalInput").ap()
    wv = nc.dram_tensor("wv", [C, HD], BF16, kind="ExternalInput").ap()
    wo = nc.dram_tensor("wo", [HD, C], BF16, kind="ExternalInput").ap()
    wg = nc.dram_tensor("wg", [C, C], BF16, kind="ExternalInput").ap()
    browg = nc.dram_tensor("browg", [1, C], BF16, kind="ExternalInput").ap()
    browo = nc.dram_tensor("browo", [1, C], BF16, kind="ExternalInput").ap()
    out = nc.dram_tensor("out", [S, C], F32, kind="ExternalOutput").ap()

    with tile.TileContext(nc) as tc, ExitStack() as ctx:
        consts = ctx.enter_context(tc.tile_pool(name="consts", bufs=1))
        sb = ctx.enter_context(tc.tile_pool(name="sb", bufs=1))
        bias_pool = ctx.enter_context(tc.tile_pool(name="bias", bufs=3))
        exp_pool = ctx.enter_context(tc.tile_pool(name="exp", bufs=3))
        res_pool = ctx.enter_context(tc.tile_pool(name="res", bufs=6))
        ps_s = ctx.enter_context(tc.tile_pool(name="ps_s", bufs=3, space="PSUM"))
        ps_o = ctx.enter_context(tc.tile_pool(name="ps_o", bufs=1, space="PSUM"))

        # ---- constants ----
        id97 = consts.tile([97, 97], F32)
        make_identity(nc, id97[:])
        ones_r = consts.tile([1, 128], BF16)
        nc.vector.memset(ones_r[:], 1.0)
        ones_c = consts.tile([128, 1], BF16)
        nc.vector.memset(ones_c[:], 1.0)

        # ---- DMAs ordered by first consumer: gating needs wg + xq
        #      chunk 0; projections need wq/wk + the rest of x ----
        def load_w2(name, dram, m):
            t = consts.tile([128, 2 * m], BF16, tag=name, name=name + "_sb")
            nc.sync.dma_start(
                t[:].rearrange("p (j m) -> p j m", j=2),
                dram.rearrange("(j p) m -> p j m", p=128),
            )
            return t

        xq_sb = sb.tile([128, 2 * S], BF16)
        xkv_sb = sb.tile([128, 2 * S], BF16)

        def load_x(t_, dram, n):
            dst = t_[:].rearrange("p (j s) -> p j s", j=2)
            src = dram.rearrange("(j p) s -> p j s", p=128)
            nc.sync.dma_start(
                dst[:, :, ds(n * 512, 512)], src[:, :, ds(n * 512, 512)]
            )

        wg_sb = load_w2("wg", wg, C)
        browg_sb = consts.tile([1, C], BF16)
        nc.sync.dma_start(browg_sb[:], browg)
        browo_sb = consts.tile([1, C], BF16)
        nc.sync.dma_start(browo_sb[:], browo)
        load_x(xq_sb, xq, 0)
        wq_sb = load_w2("wq", wq, HD)
        wk_sb = load_w2("wk", wk, HD)
        wv_sb = load_w2("wv", wv, HD)
        load_x(xkv_sb, xkv, 0)
        for n in range(1, NCH):
            load_x(xq_sb, xq, n)
            load_x(xkv_sb, xkv, n)
        wo_sb = consts.tile([HD, C], BF16)
        nc.sync.dma_start(wo_sb[:], wo)

        # ---- projections: qT, kT [HD, S] bf16 (d on partitions),
        #      col-packed: qT in rows 0-63, kT in rows 64-127 ----
        qT = sb.tile([HD, S], BF16)
        kT = sb.tile([HD, S], BF16)
        for n in range(NCH):
            ps = ps_s.tile([128, 512], F32, tag="ps", name="ps_proj")
            for j in range(2):
                nc.tensor.matmul(
                    ps[ds(0, HD), :],
                    wq_sb[:, ds(j * HD, HD)],
                    xq_sb[:, ds(j * S + n * 512, 512)],
                    start=(j == 0), stop=(j == 1),
                    tile_position=(0, 0),
                )
                nc.tensor.matmul(
                    ps[ds(HD, HD), :],
                    wk_sb[:, ds(j * HD, HD)],
                    xkv_sb[:, ds(j * S + n * 512, 512)],
                    start=(j == 0), stop=(j == 1),
                    tile_position=(0, 64),
                )
            nc.vector.tensor_copy(qT[:, ds(n * 512, 512)], ps[ds(0, HD), :])
            nc.vector.tensor_copy(kT[:, ds(n * 512, 512)], ps[ds(HD, HD), :])

        # ---- gating: g = sigmoid(xq.T @ WgT + brow_g); first half in the
        #      prologue, second half in the tail (PE is idle there) ----
        g_all = sb.tile([128, QT * C], F32)

        def emit_gate(t):
            psg = ps_s.tile([128, C], F32, tag="ps", name="psg")
            for j in range(2):
                nc.tensor.matmul(
                    psg[:],
                    xq_sb[:, ds(j * S + t * 128, 128)],
                    wg_sb[:, ds(j * C, C)],
                    start=(j == 0), stop=False,
                )
            nc.tensor.matmul(psg[:], ones_r[:], browg_sb[:], start=False, stop=True)
            nc.scalar.activation(g_all[:, ds(t * C, C)], psg[:], SIGF)

        for t in range(QT // 2):
            emit_gate(t)

        # ---- b_out/4 broadcast to [128, C] ----
        bout_bc = consts.tile([128, C], F32)
        ps_b = ps_s.tile([128, C], F32, tag="ps")
        nc.tensor.matmul(ps_b[:], ones_r[:], browo_sb[:], start=True, stop=True)
        nc.vector.tensor_copy(bout_bc[:], ps_b[:])

        # ---- V natural layout: v_sb[:, kt*HD ...]; tiles are emitted
        #      lazily (2-kt lookahead) inside the score loop ----
        v_sb = sb.tile([128, QT * HPC * D], BF16)

        def emit_v(t):
            ps = ps_s.tile([128, HD], F32, tag="ps", name="ps_v")
            for j in range(2):
                nc.tensor.matmul(
                    ps[:],
                    xkv_sb[:, ds(j * S + t * 128, 128)],
                    wv_sb[:, ds(j * HD, HD)],
                    start=(j == 0),
                    stop=(j == 1),
                )
            nc.vector.tensor_copy(v_sb[:, ds(t * HD, HD)], ps[:])

        emit_v(0)
        emit_v(1)

        # ---- attention per head, software-pipelined (PV lags QK by 1 kt);
        #      head-0's epilogue half is spread through head-1's loop ----
        # folded layouts: oT_fold rows 32n = oT[0:32, 512n:512n+512]
        oT_sb = sb.tile([128, HPC * 512], BF16)
        oT_unf = sb.tile([HD, S], BF16)  # head i rows 32i, natural q columns
        den_sb = sb.tile([97, HPC * 512], F32)
        r97 = sb.tile([128, HPC * NCH * 97], F32)
        res0_all = sb.tile([128, QT * C], F32)
        oT_tiles = {}
        den_tiles = {}

        def emit_den_chain(i):
            """den psum -> den_sb -> transposed reciprocals in r97."""
            nc.vector.tensor_copy(den_sb[:, ds(i * 512, 512)], den_tiles[i][:])
            for c in range(NCH):
                trp = ps_s.tile([128, 97], F32, tag="ps", name="trp")
                nc.tensor.transpose(
                    trp[:], den_sb[:, ds(i * 512 + c * 128, 128)], id97[:]
                )
                nc.vector.reciprocal(
                    r97[:, ds((i * NCH + c) * 97, 97)][:, 0:97:32],
                    trp[:, 0:97:32],
                )

        def emit_oproj(i, t):
            """out-projection of head i for q-tile t, scaled by 1/den."""
            n, c = t // 4, t % 4
            ps = ps_s.tile([128, C], F32, tag="ps", name="ps_op")
            nc.tensor.matmul(
                ps[:],
                oT_unf[ds(i * D, D), ds(t * 128, 128)],
                wo_sb[ds(i * D, D), :],
                start=True, stop=True,
            )
            r_ap = r97[:, ds((i * NCH + c) * 97 + 32 * n, 1)]
            if i == 0:
                res0 = res0_all[:, ds(t * C, C)]
                nc.scalar.activation(res0, ps[:], COPYF, scale=r_ap)
                nc.vector.tensor_add(res0, res0, bout_bc[:])
            else:
                if t >= QT // 2:
                    emit_gate(t)
                tmp = res_pool.tile([128, C], F32, tag="tmp", name="tmp")
                nc.scalar.activation(tmp[:], ps[:], COPYF, scale=r_ap)
                res = res_pool.tile([128, C], F32, tag="res", name="res")
                nc.vector.tensor_add(res[:], res0_all[:, ds(t * C, C)], tmp[:])
                nc.vector.tensor_mul(res[:], res[:], g_all[:, ds(t * C, C)])
                nc.sync.dma_start(out[ds(t * 128, 128), :], res[:])

        prev = None
        for j in range(HPC * QT + 1):
            cur = None
            if j < HPC * QT:
                i, kt = j // QT, j % QT
                if kt == 0:
                    oT_tiles[i] = ps_o.tile(
                        [128, 512], F32, tag="ps_oT", name="oT_ps"
                    )
                    den_tiles[i] = ps_o.tile(
                        [97, 512], F32, tag="ps_den", name="den_ps"
                    )
                bt = bias_pool.tile([128, S], BF16)
                nc.sync.dma_start(bt[:], biasT[i, ds(kt * 128, 128), :])
                et = exp_pool.tile([128, S], BF16)
                pss = []
                for n in range(2):
                    ps = ps_s.tile([128, 1024], F32, tag="ps")
                    for m in range(2):
                        nc.tensor.matmul(
                            ps[:, ds(m * 512, 512)],
                            kT[ds(i * D, D), ds(kt * 128, 128)],
                            qT[ds(i * D, D), ds(n * 1024 + m * 512, 512)],
                            start=True, stop=True,
                        )
                    pss.append(ps)
                for n in range(2):
                    nc.scalar.activation(et[:, ds(n * 1024, 1024)], pss[n][:], EXPF)
                    nc.vector.tensor_mul(
                        et[:, ds(n * 1024, 1024)],
                        et[:, ds(n * 1024, 1024)],
                        bt[:, ds(n * 1024, 1024)],
                    )
                if i == 0 and kt + 2 < QT:
                    emit_v(kt + 2)
                cur = (i, kt, et)
            if prev is not None:
                pi, pkt, pet = prev
                st, sp = pkt == 0, pkt == QT - 1
                for n in range(NCH):
                    nc.tensor.matmul(
                        oT_tiles[pi][ds(32 * n, 32), :],
                        v_sb[:, ds((pkt * HPC + pi) * D, D)],
                        pet[:, ds(n * 512, 512)],
                        start=st, stop=sp,
                        tile_position=(0, 32 * n),
                    )
                for n in range(NCH):
                    nc.tensor.matmul(
                        den_tiles[pi][ds(32 * n, 1), :],
                        ones_c[:],
                        pet[:, ds(n * 512, 512)],
                        start=st, stop=sp,
                        tile_position=(0, 32 * n),
                    )
                if sp:
                    nc.vector.tensor_copy(
                        oT_sb[:, ds(pi * 512, 512)], oT_tiles[pi][:]
                    )
                    # unfold: oT_unf[32i+p, 512n+j] = oT_sb[32n+p, 512i+j]
                    for n in range(NCH):
                        nc.scalar.dma_start(
                            oT_unf[ds(pi * D, D), ds(n * 512, 512)],
                            oT_sb[ds(32 * n, 32), ds(pi * 512, 512)],
                        )
            # spread head-0 epilogue through head-1's score loop
            if j == QT + 1:
                emit_den_chain(0)
            elif QT + 2 <= j <= QT + 1 + QT // 2:
                base = 2 * (j - QT - 2)
                emit_oproj(0, base)
                emit_oproj(0, base + 1)
            prev = cur

        # ---- tail: head-1 epilogue ----
        emit_den_chain(1)
        for t in range(QT):
            emit_oproj(1, t)

    nc.compile()
    return nc


def _shard_inputs(q_x, kv_x, attn_bias, Wq, Wk, Wv, Wout, b_out, Wg, b_g, gating_bias):
    bf = ml_dtypes.bfloat16
    in_maps = []
    scale = np.float32(D) ** np.float32(-0.5)
    for core in range(NCORES):
        b, hp = core // 4, core % 4
        hsl = slice(hp * HD, (hp + 1) * HD)
        in_maps.append(
            {
                "xq": np.ascontiguousarray(q_x[b].T).astype(bf),
                "xkv": np.ascontiguousarray(kv_x[b].T).astype(bf),
                "biasT": np.exp(
                    np.ascontiguousarray(
                        attn_bias[b, 2 * hp : 2 * hp + 2].transpose(0, 2, 1)
                    )
                ).astype(bf),
                "wq": np.ascontiguousarray((Wq[hsl] * scale).T).astype(bf),
                "wk": np.ascontiguousarray(Wk[hsl].T).astype(bf),
                "wv": np.ascontiguousarray(Wv[hsl].T).astype(bf),
                "wo": np.ascontiguousarray(Wout[:, hsl].T).astype(bf),
                "wg": np.ascontiguousarray(Wg.T).astype(bf),
                "browg": (b_g + gating_bias).reshape(1, C).astype(bf),
                "browo": (b_out / 4.0).reshape(1, C).astype(bf),
            }
        )
    return in_maps


def run(inputs, trace=False, **kw):
    if "nc" not in _NC_CACHE:
        _NC_CACHE["nc"] = build_nc()
    nc = _NC_CACHE["nc"]
    inputs = {k: np.asarray(v, dtype=np.float32) for k, v in inputs.items()}
    in_maps = _shard_inputs(**inputs)
    r = run_bass_kernel_spmd(nc, in_maps, core_ids=list(range(NCORES)), trace=trace, **kw)
    outs = np.stack([np.asarray(m["out"], np.float32) for m in r.results])
    full = outs.reshape(B, 4, S, C).sum(axis=1)
    return full, r


def kernel(**inputs) -> np.ndarray:
    full, _ = run(inputs, trace=False)
    return full


if __name__ == "__main__":
    print("building...")
    build_nc()
    print("ok")


# revision 57
# speedup vs baseline: 1.0505x; 1.0279x over previous
"""Distributed Bass kernel for nn_Attention_20993800143414.

Reference computation (B=2, S=2048, C=256, H=8, D=32):
    q = (q_x @ Wq.T) * D**-0.5 ; k = kv_x @ Wk.T ; v = kv_x @ Wv.T
    scores = einsum("bqhd,bkhd->bhqk", q, k) + attn_bias
    w = softmax(scores, -1)
    o = einsum("bhqk,bkhd->bqhd", w, v).reshape(b, s, C) @ Wout.T + b_out
    out = o * sigmoid(q_x @ Wg.T + b_g + gating_bias)

Sharding: 16 (b,h) pairs -> 8 cores, 2 heads of one batch per core.
Each core computes its 2 heads' attention + the partial out-projection
(+ b_out/4 rank-1 term) multiplied by the gating; the host sums the
4 partials per batch (pure unshard-reduce for head parallelism).

Layout: scores are built TRANSPOSED, sT[k,q] (k on partitions):
  - host ships exp(biasT) bf16; after ACT exp of the qk scores the bias
    becomes a DVE bf16 multiply (exp(s+b) = exp(s)*exp(b)).
  - PV is col-group packed: 4 concurrent M=32 matmuls (tile_position
    (0,32n)) write a FOLDED oT psum [128,512] (rows 32n = q-chunk n),
    one PSUM bank, ~1 stream-time per k-tile.
  - softmax denominators: 4 concurrent M=1 matmuls with a ones vector
    into den_ps[97,512] rows 32n.
  - out-projection consumes folded oT directly as lhsT; 1/denominator
    is applied afterwards as a per-partition scalar.
  - the PV/den wave lags QK by one k-tile (software pipeline) so the
    PE stream stays dense.
"""

import sys

for _p in ("/opt/trn_rl_repo",):
    if _p not in sys.path:
        sys.path.insert(0, _p)

import numpy as np
import ml_dtypes
from contextlib import ExitStack

import concourse.bass as bass
import concourse.bacc as bacc
import concourse.mybir as mybir
import concourse.tile as tile
from concourse.bass import ds
from concourse.bass_utils import run_bass_kernel_spmd
from concourse.masks import make_identity

B, S, C, H, D = 2, 2048, 256, 8, 32
NCORES = 8
HPC = (B * H) // NCORES  # heads per core = 2
HD = HPC * D  # 64
QT = S // 128  # 16 q (and k) tiles
NCH = S // 512  # 4 free-dim chunks of 512
BF16 = mybir.dt.bfloat16
F32 = mybir.dt.float32
EXPF = mybir.ActivationFunctionType.Exp
SIGF = mybir.ActivationFunctionType.Sigmoid
COPYF = mybir.ActivationFunctionType.Copy

_NC_CACHE = {}


def build_nc():
    nc = bacc.Bacc("TRN2", target_bir_lowering=False, debug=False, num_devices=NCORES)

    xq = nc.dram_tensor("xq", [C, S], BF16, kind="ExternalInput").ap()
    xkv = nc.dram_tensor("xkv", [C, S], BF16, kind="ExternalInput").ap()
    biasT = nc.dram_tensor("biasT", [HPC, S, S], BF16, kind="ExternalInput").ap()
    wq = nc.dram_tensor("wq", [C, HD], BF16, kind="ExternalInput").ap()
    wk = nc.dram_tensor("wk", [C, HD], BF16, kind="ExternalInput").ap()
    wv = nc.dram_tensor("wv", [C, HD], BF16, kind="ExternalInput").ap()
    wo = nc.dram_tensor("wo", [HD, C], BF16, kind="ExternalInput").ap()
    wg = nc.dram_tensor("wg", [C, C], BF16, kind="ExternalInput").ap()
    browg = nc.dram_tensor("browg", [1, C], BF16, kind="ExternalInput").ap()
    browo = nc.dram_tensor("browo", [1, C], BF16, kind="ExternalInput").ap()
    out = nc.dram_tensor("out", [S, C], F32, kind="ExternalOutput").ap()

    with tile.TileContext(nc) as tc, ExitStack() as ctx:
        consts = ctx.enter_context(tc.tile_pool(name="consts", bufs=1))
        sb = ctx.enter_context(tc.tile_pool(name="sb", bufs=1))
        bias_pool = ctx.enter_context(tc.tile_pool(name="bias", bufs=3))
        exp_pool = ctx.enter_context(tc.tile_pool(name="exp", bufs=3))
        res_pool = ctx.enter_context(tc.tile_pool(name="res", bufs=6))
        ps_s = ctx.enter_context(tc.tile_pool(name="ps_s", bufs=3, space="PSUM"))
        ps_o = ctx.enter_context(tc.tile_pool(name="ps_o", bufs=1, space="PSUM"))

        # ---- constants ----
        id97 = consts.tile([97, 97], F32)
        make_identity(nc, id97[:])
        ones_r = consts.tile([1, 128], BF16)
        nc.vector.memset(ones_r[:], 1.0)
        ones_c = consts.tile([128, 1], BF16)
        nc.vector.memset(ones_c[:], 1.0)

        # ---- DMAs ordered by first consumer: gating needs wg + xq
        #      chunk 0; projections need wq/wk + the rest of x ----
        def load_w2(name, dram, m):
            t = consts.tile([128, 2 * m], BF16, tag=name, name=name + "_sb")
            nc.sync.dma_start(
                t[:].rearrange("p (j m) -> p j m", j=2),
                dram.rearrange("(j p) m -> p j m", p=128),
            )
            return t

        xq_sb = sb.tile([128, 2 * S], BF16)
        xkv_sb = sb.tile([128, 2 * S], BF16)

        def load_x(t_, dram, n):
            dst = t_[:].rearrange("p (j s) -> p j s", j=2)
            src = dram.rearrange("(j p) s -> p j s", p=128)
            nc.sync.dma_start(
                dst[:, :, ds(n * 512, 512)], src[:, :, ds(n * 512, 512)]
            )

        wg_sb = load_w2("wg", wg, C)
        browg_sb = consts.tile([1, C], BF16)
        nc.sync.dma_start(browg_sb[:], browg)
        browo_sb = consts.tile([1, C], BF16)
        nc.sync.dma_start(browo_sb[:], browo)
        load_x(xq_sb, xq, 0)
        wq_sb = load_w2("wq", wq, HD)
        wk_sb = load_w2("wk", wk, HD)
        wv_sb = load_w2("wv", wv, HD)
        load_x(xkv_sb, xkv, 0)
        for n in range(1, NCH):
            load_x(xq_sb, xq, n)
            load_x(xkv_sb, xkv, n)
        wo_sb = consts.tile([HD, C], BF16)
        nc.sync.dma_start(wo_sb[:], wo)

        # ---- projections: qT, kT [HD, S] bf16 (d on partitions),
        #      col-packed: qT in rows 0-63, kT in rows 64-127 ----
        qT = sb.tile([HD, S], BF16)
        kT = sb.tile([HD, S], BF16)
        for n in range(NCH):
            ps = ps_s.tile([128, 512], F32, tag="ps", name="ps_proj")
            for j in range(2):
                nc.tensor.matmul(
                    ps[ds(0, HD), :],
                    wq_sb[:, ds(j * HD, HD)],
                    xq_sb[:, ds(j * S + n * 512, 512)],
                    start=(j == 0), stop=(j == 1),
                    tile_position=(0, 0),
                )
                nc.tensor.matmul(
                    ps[ds(HD, HD), :],
                    wk_sb[:, ds(j * HD, HD)],
                    xkv_sb[:, ds(j * S + n * 512, 512)],
                    start=(j == 0), stop=(j == 1),
                    tile_position=(0, 64),
                )
            nc.vector.tensor_copy(qT[:, ds(n * 512, 512)], ps[ds(0, HD), :])
            nc.vector.tensor_copy(kT[:, ds(n * 512, 512)], ps[ds(HD, HD), :])

        # ---- gating: g = sigmoid(xq.T @ WgT + brow_g); first half in the
        #      prologue, second half in the tail (PE is idle there) ----
        g_all = sb.tile([128, QT * C], F32)

        def emit_gate(t):
            psg = ps_s.tile([128, C], F32, tag="ps", name="psg")
            for j in range(2):
                nc.tensor.matmul(
                    psg[:],
                    xq_sb[:, ds(j * S + t * 128, 128)],
                    wg_sb[:, ds(j * C, C)],
                    start=(j == 0), stop=False,
                )
            nc.tensor.matmul(psg[:], ones_r[:], browg_sb[:], start=False, stop=True)
            nc.scalar.activation(g_all[:, ds(t * C, C)], psg[:], SIGF)

        for t in range(QT // 2):
            emit_gate(t)

        # ---- b_out/4 broadcast to [128, C] ----
        bout_bc = consts.tile([128, C], F32)
        ps_b = ps_s.tile([128, C], F32, tag="ps")
        nc.tensor.matmul(ps_b[:], ones_r[:], browo_sb[:], start=True, stop=True)
        nc.vector.tensor_copy(bout_bc[:], ps_b[:])

        # ---- V natural layout: v_sb[:, kt*HD ...]; tiles are emitted
        #      lazily (2-kt lookahead) inside the score loop ----
        v_sb = sb.tile([128, QT * HPC * D], BF16)

        def emit_v(t):
            ps = ps_s.tile([128, HD], F32, tag="ps", name="ps_v")
            for j in range(2):
                nc.tensor.matmul(
                    ps[:],
                    xkv_sb[:, ds(j * S + t * 128, 128)],
                    wv_sb[:, ds(j * HD, HD)],
                    start=(j == 0),
                    stop=(j == 1),
                )
            nc.vector.tensor_copy(v_sb[:, ds(t * HD, HD)], ps[:])

        emit_v(0)
        emit_v(1)

        # ---- attention per head, software-pipelined (PV lags QK by 1 kt);
        #      head-0's epilogue half is spread through head-1's loop ----
        # folded layouts: oT_fold rows 32n = oT[0:32, 512n:512n+512]
        oT_sb = sb.tile([128, HPC * 512], BF16)
        oT_unf = sb.tile([HD, S], BF16)  # head i rows 32i, natural q columns
        den_sb = sb.tile([97, HPC * 512], F32)
        r97 = sb.tile([128, HPC * NCH * 97], F32)
        res0_all = sb.tile([128, QT * C], F32)
        oT_tiles = {}
        den_tiles = {}

        def emit_den_chain(i):
            """den psum -> den_sb -> transposed reciprocals in r97."""
            nc.vector.tensor_copy(den_sb[:, ds(i * 512, 512)], den_tiles[i][:])
            for c in range(NCH):
                trp = ps_s.tile([128, 97], F32, tag="ps", name="trp")
                nc.tensor.transpose(
                    trp[:], den_sb[:, ds(i * 512 + c * 128, 128)], id97[:]
                )
                nc.vector.reciprocal(
                    r97[:, ds((i * NCH + c) * 97, 97)][:, 0:97:32],
                    trp[:, 0:97:32],
                )

        def emit_oproj(i, t):
            """out-projection of head i for q-tile t, scaled by 1/den."""
            n, c = t // 4, t % 4
            ps = ps_s.tile([128, C], F32, tag="ps", name="ps_op")
            nc.tensor.matmul(
                ps[:],
                oT_unf[ds(i * D, D), ds(t * 128, 128)],
                wo_sb[ds(i * D, D), :],
                start=True, stop=True,
            )
            r_ap = r97[:, ds((i * NCH + c) * 97 + 32 * n, 1)]
            if i == 0:
                res0 = res0_all[:, ds(t * C, C)]
                nc.scalar.activation(res0, ps[:], COPYF, scale=r_ap)
                nc.vector.tensor_add(res0, res0, bout_bc[:])
            else:
                if t >= QT // 2:
                    emit_gate(t)
                tmp = res_pool.tile([128, C], F32, tag="tmp", name="tmp")
                nc.scalar.activation(tmp[:], ps[:], COPYF, scale=r_ap)
                res = res_pool.tile([128, C], F32, tag="res", name="res")
                nc.vector.tensor_add(res[:], res0_all[:, ds(t * C, C)], tmp[:])
                nc.vector.tensor_mul(res[:], res[:], g_all[:, ds(t * C, C)])
                nc.sync.dma_start(out[ds(t * 128, 128), :], res[:])

        prev = None
        for j in range(HPC * QT + 1):
            cur = None
            if j < HPC * QT:
                i, kt = j // QT, j % QT
                if kt == 0:
                    oT_tiles[i] = ps_o.tile(
                        [128, 512], F32, tag="ps_oT", name="oT_ps"
                    )
                    den_tiles[i] = ps_o.tile(
                        [97, 512], F32, tag="ps_den", name="den_ps"
                    )
                bt = bias_pool.tile([128, S], BF16)
                nc.sync.dma_start(bt[:], biasT[i, ds(kt * 128, 128), :])
                et = exp_pool.tile([128, S], BF16)
                pss = []
                for n in range(2):
                    ps = ps_s.tile([128, 1024], F32, tag="ps")
                    for m in range(2):
                        nc.tensor.matmul(
                            ps[:, ds(m * 512, 512)],
                            kT[ds(i * D, D), ds(kt * 128, 128)],
                            qT[ds(i * D, D), ds(n * 1024 + m * 512, 512)],
                            start=True, stop=True,
                        )
                    pss.append(ps)
                for n in range(2):
                    nc.scalar.activation(et[:, ds(n * 1024, 1024)], pss[n][:], EXPF)
                    nc.vector.tensor_mul(
                        et[:, ds(n * 1024, 1024)],
                        et[:, ds(n * 1024, 1024)],
                        bt[:, ds(n * 1024, 1024)],
                    )
                if i == 0 and kt + 2 < QT:
                    emit_v(kt + 2)
                cur = (i, kt, et)
            if prev is not None:
                pi, pkt, pet = prev
                st, sp = pkt == 0, pkt == QT - 1
                for n in range(NCH):
                    nc.tensor.matmul(
                        oT_tiles[pi][ds(32 * n, 32), :],
                        v_sb[:, ds((pkt * HPC + pi) * D, D)],
                        pet[:, ds(n * 512, 512)],
                        start=st, stop=sp,
                        tile_position=(0, 32 * n),
                    )
                for n in range(NCH):
                    nc.tensor.matmul(
                        den_tiles[pi][ds(32 * n, 1), :],
                        ones_c[:],
                        pet[:, ds(n * 512, 512)],
                        start=st, stop=sp,
                        tile_position=(0, 32 * n),
                    )
                if sp:
                    nc.vector.tensor_copy(
                        oT_sb[:, ds(pi * 512, 512)], oT_tiles[pi][:]
                    )
                    # unfold: oT_unf[32i+p, 512n+j] = oT_sb[32n+p, 512i+j]
                    for n in range(NCH):
                        nc.scalar.dma_start(
                            oT_unf[ds(pi * D, D), ds(n * 512, 512)],
                            oT_sb[ds(32 * n, 32), ds(pi * 512, 512)],
                        )
            # spread head-0 epilogue through head-1's score loop
            if j == QT + 1:
                emit_den_chain(0)
            elif QT + 2 <= j <= QT + 1 + QT // 2:
                base = 2 * (j - QT - 2)
                emit_oproj(0, base)
                emit_oproj(0, base + 1)
            prev = cur

        # ---- tail: head-1 epilogue ----
        emit_den_chain(1)
        for t in range(QT):
            emit_oproj(1, t)

    nc.compile()
    return nc


def _shard_inputs(q_x, kv_x, attn_bias, Wq, Wk, Wv, Wout, b_out, Wg, b_g, gating_bias):
    bf = ml_dtypes.bfloat16
    in_maps = []
    scale = np.float32(D) ** np.float32(-0.5)
    for core in range(NCORES):
        b, hp = core // 4, core % 4
        hsl = slice(hp * HD, (hp + 1) * HD)
        in_maps.append(
            {
                "xq": np.ascontiguousarray(q_x[b].T).astype(bf),
                "xkv": np.ascontiguousarray(kv_x[b].T).astype(bf),
                "biasT": np.exp(
                    np.ascontiguousarray(
                        attn_bias[b, 2 * hp : 2 * hp + 2].transpose(0, 2, 1)
                    )
                ).astype(bf),
                "wq": np.ascontiguousarray((Wq[hsl] * scale).T).astype(bf),
                "wk": np.ascontiguousarray(Wk[hsl].T).astype(bf),
                "wv": np.ascontiguousarray(Wv[hsl].T).astype(bf),
                "wo": np.ascontiguousarray(Wout[:, hsl].T).astype(bf),
                "wg": np.ascontiguousarray(Wg.T).astype(bf),
                "browg": (b_g + gating_bias).reshape(1, C).astype(bf),
                "browo": (b_out / 4.0).reshape(1, C).astype(bf),
            }
        )
    return in_maps


def run(inputs, trace=False, **kw):
    if "nc" not in _NC_CACHE:
        _NC_CACHE["nc"] = build_nc()
    nc = _NC_CACHE["nc"]
    inputs = {k: np.asarray(v, dtype=np.float32) for k, v in inputs.items()}
    in_maps = _shard_inputs(**inputs)
    r = run_bass_kernel_spmd(nc, in_maps, core_ids=list(range(NCORES)), trace=trace, **kw)
    outs = np.stack([np.asarray(m["out"], np.float32) for m in r.results])
    full = outs.reshape(B, 4, S, C).sum(axis=1)
    return full, r


def kernel(**inputs) -> np.ndarray:
    full, _ = run(inputs, trace=False)
    return full


if __name__ == "__main__":
    print("building...")
    build_nc()
    print("ok")


# revision 59
# speedup vs baseline: 1.0599x; 1.0089x over previous
"""Distributed Bass kernel for nn_Attention_20993800143414.

Reference computation (B=2, S=2048, C=256, H=8, D=32):
    q = (q_x @ Wq.T) * D**-0.5 ; k = kv_x @ Wk.T ; v = kv_x @ Wv.T
    scores = einsum("bqhd,bkhd->bhqk", q, k) + attn_bias
    w = softmax(scores, -1)
    o = einsum("bhqk,bkhd->bqhd", w, v).reshape(b, s, C) @ Wout.T + b_out
    out = o * sigmoid(q_x @ Wg.T + b_g + gating_bias)

Sharding: 16 (b,h) pairs -> 8 cores, 2 heads of one batch per core.
Each core computes its 2 heads' attention + the partial out-projection
(+ b_out/4 rank-1 term) multiplied by the gating; the host sums the
4 partials per batch (pure unshard-reduce for head parallelism).

Layout: scores are built TRANSPOSED, sT[k,q] (k on partitions):
  - host ships exp(biasT) bf16; after ACT exp of the qk scores the bias
    becomes a DVE bf16 multiply (exp(s+b) = exp(s)*exp(b)).
  - PV is col-group packed: 4 concurrent M=32 matmuls (tile_position
    (0,32n)) write a FOLDED oT psum [128,512] (rows 32n = q-chunk n),
    one PSUM bank, ~1 stream-time per k-tile.
  - softmax denominators: 4 concurrent M=1 matmuls with a ones vector
    into den_ps[97,512] rows 32n.
  - out-projection consumes folded oT directly as lhsT; 1/denominator
    is applied afterwards as a per-partition scalar.
  - the PV/den wave lags QK by one k-tile (software pipeline) so the
    PE stream stays dense.
"""

import sys

for _p in ("/opt/trn_rl_repo",):
    if _p not in sys.path:
        sys.path.insert(0, _p)

import numpy as np
import ml_dtypes
from contextlib import ExitStack

import concourse.bass as bass
import concourse.bacc as bacc
import concourse.mybir as mybir
import concourse.tile as tile
from concourse.bass import ds
from concourse.bass_utils import run_bass_kernel_spmd
from concourse.masks import make_identity

B, S, C, H, D = 2, 2048, 256, 8, 32
NCORES = 8
HPC = (B * H) // NCORES  # heads per core = 2
HD = HPC * D  # 64
QT = S // 128  # 16 q (and k) tiles
NCH = S // 512  # 4 free-dim chunks of 512
BF16 = mybir.dt.bfloat16
F32 = mybir.dt.float32
EXPF = mybir.ActivationFunctionType.Exp
SIGF = mybir.ActivationFunctionType.Sigmoid
COPYF = mybir.ActivationFunctionType.Copy

_NC_CACHE = {}


def build_nc():
    nc = bacc.Bacc("TRN2", target_bir_lowering=False, debug=False, num_devices=NCORES)

    xq = nc.dram_tensor("xq", [C, S], BF16, kind="ExternalInput").ap()
    xkv = nc.dram_tensor("xkv", [C, S], BF16, kind="ExternalInput").ap()
    biasT = nc.dram_tensor("biasT", [HPC, S, S], BF16, kind="ExternalInput").ap()
    wq = nc.dram_tensor("wq", [C, HD], BF16, kind="ExternalInput").ap()
    wk = nc.dram_tensor("wk", [C, HD], BF16, kind="ExternalInput").ap()
    wv = nc.dram_tensor("wv", [C, HD], BF16, kind="ExternalInput").ap()
    wo = nc.dram_tensor("wo", [HD, C], BF16, kind="ExternalInput").ap()
    wg = nc.dram_tensor("wg", [C, C], BF16, kind="ExternalInput").ap()
    browg = nc.dram_tensor("browg", [1, C], BF16, kind="ExternalInput").ap()
    browo = nc.dram_tensor("browo", [1, C], BF16, kind="ExternalInput").ap()
    out = nc.dram_tensor("out", [S, C], F32, kind="ExternalOutput").ap()

    with tile.TileContext(nc) as tc, ExitStack() as ctx:
        consts = ctx.enter_context(tc.tile_pool(name="consts", bufs=1))
        sb = ctx.enter_context(tc.tile_pool(name="sb", bufs=1))
        bias_pool = ctx.enter_context(tc.tile_pool(name="bias", bufs=3))
        exp_pool = ctx.enter_context(tc.tile_pool(name="exp", bufs=3))
        res_pool = ctx.enter_context(tc.tile_pool(name="res", bufs=6))
        ps_s = ctx.enter_context(tc.tile_pool(name="ps_s", bufs=3, space="PSUM"))
        ps_o = ctx.enter_context(tc.tile_pool(name="ps_o", bufs=1, space="PSUM"))

        # ---- constants ----
        id97 = consts.tile([97, 97], F32)
        make_identity(nc, id97[:])
        ones_r = consts.tile([1, 128], BF16)
        nc.vector.memset(ones_r[:], 1.0)
        ones_c = consts.tile([128, 1], BF16)
        nc.vector.memset(ones_c[:], 1.0)

        # ---- DMAs ordered by first consumer: gating needs wg + xq
        #      chunk 0; projections need wq/wk + the rest of x ----
        def load_w2(name, dram, m):
            t = consts.tile([128, 2 * m], BF16, tag=name, name=name + "_sb")
            nc.sync.dma_start(
                t[:].rearrange("p (j m) -> p j m", j=2),
                dram.rearrange("(j p) m -> p j m", p=128),
            )
            return t

        xq_sb = sb.tile([128, 2 * S], BF16)
        xkv_sb = sb.tile([128, 2 * S], BF16)

        def load_x(t_, dram, n):
            dst = t_[:].rearrange("p (j s) -> p j s", j=2)
            src = dram.rearrange("(j p) s -> p j s", p=128)
            nc.sync.dma_start(
                dst[:, :, ds(n * 512, 512)], src[:, :, ds(n * 512, 512)]
            )

        wg_sb = load_w2("wg", wg, C)
        browg_sb = consts.tile([1, C], BF16)
        nc.sync.dma_start(browg_sb[:], browg)
        browo_sb = consts.tile([1, C], BF16)
        nc.sync.dma_start(browo_sb[:], browo)
        load_x(xq_sb, xq, 0)
        wq_sb = load_w2("wq", wq, HD)
        wk_sb = load_w2("wk", wk, HD)
        wv_sb = load_w2("wv", wv, HD)
        load_x(xkv_sb, xkv, 0)
        for n in range(1, NCH):
            load_x(xq_sb, xq, n)
            load_x(xkv_sb, xkv, n)
        wo_sb = consts.tile([HD, C], BF16)
        nc.sync.dma_start(wo_sb[:], wo)

        # ---- projections: qT, kT [HD, S] bf16 (d on partitions),
        #      col-packed: qT in rows 0-63, kT in rows 64-127 ----
        qT = sb.tile([HD, S], BF16)
        kT = sb.tile([HD, S], BF16)
        for n in range(NCH):
            ps = ps_s.tile([128, 512], F32, tag="ps", name="ps_proj")
            for j in range(2):
                nc.tensor.matmul(
                    ps[ds(0, HD), :],
                    wq_sb[:, ds(j * HD, HD)],
                    xq_sb[:, ds(j * S + n * 512, 512)],
                    start=(j == 0), stop=(j == 1),
                    tile_position=(0, 0),
                )
                nc.tensor.matmul(
                    ps[ds(HD, HD), :],
                    wk_sb[:, ds(j * HD, HD)],
                    xkv_sb[:, ds(j * S + n * 512, 512)],
                    start=(j == 0), stop=(j == 1),
                    tile_position=(0, 64),
                )
            nc.vector.tensor_copy(qT[:, ds(n * 512, 512)], ps[ds(0, HD), :])
            nc.vector.tensor_copy(kT[:, ds(n * 512, 512)], ps[ds(HD, HD), :])

        # ---- gating: g = sigmoid(xq.T @ WgT + brow_g); first half in the
        #      prologue, second half in the tail (PE is idle there) ----
        g_all = sb.tile([128, QT * C], F32)

        def emit_gate(t):
            psg = ps_s.tile([128, C], F32, tag="ps", name="psg")
            for j in range(2):
                nc.tensor.matmul(
                    psg[:],
                    xq_sb[:, ds(j * S + t * 128, 128)],
                    wg_sb[:, ds(j * C, C)],
                    start=(j == 0), stop=False,
                )
            nc.tensor.matmul(psg[:], ones_r[:], browg_sb[:], start=False, stop=True)
            nc.scalar.activation(g_all[:, ds(t * C, C)], psg[:], SIGF)

        for t in range(QT // 2):
            emit_gate(t)

        # ---- b_out/4 broadcast to [128, C] ----
        bout_bc = consts.tile([128, C], F32)
        ps_b = ps_s.tile([128, C], F32, tag="ps")
        nc.tensor.matmul(ps_b[:], ones_r[:], browo_sb[:], start=True, stop=True)
        nc.vector.tensor_copy(bout_bc[:], ps_b[:])

        # ---- V natural layout: v_sb[:, kt*HD ...]; tiles are emitted
        #      lazily (2-kt lookahead) inside the score loop ----
        v_sb = sb.tile([128, QT * HPC * D], BF16)

        def emit_v(t):
            ps = ps_s.tile([128, HD], F32, tag="ps", name="ps_v")
            for j in range(2):
                nc.tensor.matmul(
                    ps[:],
                    xkv_sb[:, ds(j * S + t * 128, 128)],
                    wv_sb[:, ds(j * HD, HD)],
                    start=(j == 0),
                    stop=(j == 1),
                )
            nc.vector.tensor_copy(v_sb[:, ds(t * HD, HD)], ps[:])

        emit_v(0)
        emit_v(1)

        # ---- attention per head, software-pipelined (PV lags QK by 1 kt);
        #      head-0's epilogue half is spread through head-1's loop ----
        # folded layouts: oT_fold rows 32n = oT[0:32, 512n:512n+512]
        oT_sb = sb.tile([128, HPC * 512], BF16)
        oT_unf = sb.tile([HD, S], BF16)  # head i rows 32i, natural q columns
        den_sb = sb.tile([97, HPC * 512], F32)
        r97 = sb.tile([128, HPC * NCH * 97], F32)
        res0_all = sb.tile([128, QT * C], F32)
        oT_tiles = {}
        den_tiles = {}

        def emit_den_chain(i):
            """den psum -> den_sb -> transposed reciprocals in r97."""
            nc.vector.tensor_copy(den_sb[:, ds(i * 512, 512)], den_tiles[i][:])
            for c in range(NCH):
                trp = ps_s.tile([128, 97], F32, tag="ps", name="trp")
                nc.tensor.transpose(
                    trp[:], den_sb[:, ds(i * 512 + c * 128, 128)], id97[:]
                )
                nc.vector.reciprocal(
                    r97[:, ds((i * NCH + c) * 97, 97)][:, 0:97:32],
                    trp[:, 0:97:32],
                )

        def emit_oproj(i, t):
            """out-projection of head i for q-tile t, scaled by 1/den."""
            n, c = t // 4, t % 4
            ps = ps_s.tile([128, C], F32, tag="ps", name="ps_op")
            nc.tensor.matmul(
                ps[:],
                oT_unf[ds(i * D, D), ds(t * 128, 128)],
                wo_sb[ds(i * D, D), :],
                start=True, stop=True,
            )
            r_ap = r97[:, ds((i * NCH + c) * 97 + 32 * n, 1)]
            if i == 0:
                res0 = res0_all[:, ds(t * C, C)]
                nc.scalar.activation(res0, ps[:], COPYF, scale=r_ap)
                nc.vector.tensor_add(res0, res0, bout_bc[:])
            else:
                if t >= QT // 2:
                    emit_gate(t)
                tmp = res_pool.tile([128, C], F32, tag="tmp", name="tmp")
                nc.scalar.activation(tmp[:], ps[:], COPYF, scale=r_ap)
                res = res_pool.tile([128, C], F32, tag="res", name="res")
                nc.vector.tensor_add(res[:], res0_all[:, ds(t * C, C)], tmp[:])
                nc.vector.tensor_mul(res[:], res[:], g_all[:, ds(t * C, C)])
                nc.sync.dma_start(out[ds(t * 128, 128), :], res[:])

        prev = None
        for j in range(HPC * QT + 1):
            cur = None
            if j < HPC * QT:
                i, kt = j // QT, j % QT
                if kt == 0:
                    oT_tiles[i] = ps_o.tile(
                        [128, 512], F32, tag="ps_oT", name="oT_ps"
                    )
                    den_tiles[i] = ps_o.tile(
                        [97, 512], F32, tag="ps_den", name="den_ps"
                    )
                bt = bias_pool.tile([128, S], BF16)
                nc.sync.dma_start(bt[:], biasT[i, ds(kt * 128, 128), :])
                et = exp_pool.tile([128, S], BF16)
                pss = []
                for n in range(2):
                    ps = ps_s.tile([128, 1024], F32, tag="ps")
                    for m in range(2):
                        nc.tensor.matmul(
                            ps[:, ds(m * 512, 512)],
                            kT[ds(i * D, D), ds(kt * 128, 128)],
                            qT[ds(i * D, D), ds(n * 1024 + m * 512, 512)],
                            start=True, stop=True,
                        )
                    pss.append(ps)
                for n in range(2):
                    nc.scalar.activation(et[:, ds(n * 1024, 1024)], pss[n][:], EXPF)
                    nc.vector.tensor_mul(
                        et[:, ds(n * 1024, 1024)],
                        et[:, ds(n * 1024, 1024)],
                        bt[:, ds(n * 1024, 1024)],
                    )
                if i == 0 and kt + 2 < QT:
                    emit_v(kt + 2)
                cur = (i, kt, et)
            if prev is not None:
                pi, pkt, pet = prev
                st, sp = pkt == 0, pkt == QT - 1
                for n in range(NCH):
                    nc.tensor.matmul(
                        oT_tiles[pi][ds(32 * n, 32), :],
                        v_sb[:, ds((pkt * HPC + pi) * D, D)],
                        pet[:, ds(n * 512, 512)],
                        start=st, stop=sp,
                        tile_position=(0, 32 * n),
                    )
                for n in range(NCH):
                    nc.tensor.matmul(
                        den_tiles[pi][ds(32 * n, 1), :],
                        ones_c[:],
                        pet[:, ds(n * 512, 512)],
                        start=st, stop=sp,
                        tile_position=(0, 32 * n),
                    )
                if sp:
                    nc.vector.tensor_copy(
                        oT_sb[:, ds(pi * 512, 512)], oT_tiles[pi][:]
                    )
                    # unfold: oT_unf[32i+p, 512n+j] = oT_sb[32n+p, 512i+j]
                    for n in range(NCH):
                        nc.scalar.dma_start(
                            oT_unf[ds(pi * D, D), ds(n * 512, 512)],
                            oT_sb[ds(32 * n, 32), ds(pi * 512, 512)],
                        )
            # spread head-0 epilogue through head-1's score loop
            if j == QT:
                emit_den_chain(0)
            elif QT + 1 <= j <= 2 * QT:
                emit_oproj(0, j - QT - 1)
            prev = cur

        # ---- tail: head-1 epilogue ----
        emit_den_chain(1)
        for t in range(QT):
            emit_oproj(1, t)

    nc.compile()
    return nc


def _shard_inputs(q_x, kv_x, attn_bias, Wq, Wk, Wv, Wout, b_out, Wg, b_g, gating_bias):
    bf = ml_dtypes.bfloat16
    in_maps = []
    scale = np.float32(D) ** np.float32(-0.5)
    for core in range(NCORES):
        b, hp = core // 4, core % 4
        hsl = slice(hp * HD, (hp + 1) * HD)
        in_maps.append(
            {
                "xq": np.ascontiguousarray(q_x[b].T).astype(bf),
                "xkv": np.ascontiguousarray(kv_x[b].T).astype(bf),
                "biasT": np.exp(
                    np.ascontiguousarray(
                        attn_bias[b, 2 * hp : 2 * hp + 2].transpose(0, 2, 1)
                    )
                ).astype(bf),
                "wq": np.ascontiguousarray((Wq[hsl] * scale).T).astype(bf),
                "wk": np.ascontiguousarray(Wk[hsl].T).astype(bf),
                "wv": np.ascontiguousarray(Wv[hsl].T).astype(bf),
                "wo": np.ascontiguousarray(Wout[:, hsl].T).astype(bf),
                "wg": np.ascontiguousarray(Wg.T).astype(bf),
                "browg": (b_g + gating_bias).reshape(1, C).astype(bf),
                "browo": (b_out / 4.0).reshape(1, C).astype(bf),
            }
        )
    return in_maps


def run(inputs, trace=False, **kw):
    if "nc" not in _NC_CACHE:
        _NC_CACHE["nc"] = build_nc()
    nc = _NC_CACHE["nc"]
    inputs = {k: np.asarray(v, dtype=np.float32) for k, v in inputs.items()}
    in_maps = _shard_inputs(**inputs)
    r = run_bass_kernel_spmd(nc, in_maps, core_ids=list(range(NCORES)), trace=trace, **kw)
    outs = np.stack([np.asarray(m["out"], np.float32) for m in r.results])
    full = outs.reshape(B, 4, S, C).sum(axis=1)
    return full, r


def kernel(**inputs) -> np.ndarray:
    full, _ = run(inputs, trace=False)
    return full


if __name__ == "__main__":
    print("building...")
    build_nc()
    print("ok")


# revision 60
# speedup vs baseline: 1.0692x; 1.0089x over previous
"""Distributed Bass kernel for nn_Attention_20993800143414.

Reference computation (B=2, S=2048, C=256, H=8, D=32):
    q = (q_x @ Wq.T) * D**-0.5 ; k = kv_x @ Wk.T ; v = kv_x @ Wv.T
    scores = einsum("bqhd,bkhd->bhqk", q, k) + attn_bias
    w = softmax(scores, -1)
    o = einsum("bhqk,bkhd->bqhd", w, v).reshape(b, s, C) @ Wout.T + b_out
    out = o * sigmoid(q_x @ Wg.T + b_g + gating_bias)

Sharding: 16 (b,h) pairs -> 8 cores, 2 heads of one batch per core.
Each core computes its 2 heads' attention + the partial out-projection
(+ b_out/4 rank-1 term) multiplied by the gating; the host sums the
4 partials per batch (pure unshard-reduce for head parallelism).

Layout: scores are built TRANSPOSED, sT[k,q] (k on partitions):
  - host ships exp(biasT) bf16; after ACT exp of the qk scores the bias
    becomes a DVE bf16 multiply (exp(s+b) = exp(s)*exp(b)).
  - PV is col-group packed: 4 concurrent M=32 matmuls (tile_position
    (0,32n)) write a FOLDED oT psum [128,512] (rows 32n = q-chunk n),
    one PSUM bank, ~1 stream-time per k-tile.
  - softmax denominators: 4 concurrent M=1 matmuls with a ones vector
    into den_ps[97,512] rows 32n.
  - out-projection consumes folded oT directly as lhsT; 1/denominator
    is applied afterwards as a per-partition scalar.
  - the PV/den wave lags QK by one k-tile (software pipeline) so the
    PE stream stays dense.
"""

import sys

for _p in ("/opt/trn_rl_repo",):
    if _p not in sys.path:
        sys.path.insert(0, _p)

import numpy as np
import ml_dtypes
from contextlib import ExitStack

import concourse.bass as bass
import concourse.bacc as bacc
import concourse.mybir as mybir
import concourse.tile as tile
from concourse.bass import ds
from concourse.bass_utils import run_bass_kernel_spmd
from concourse.masks import make_identity

B, S, C, H, D = 2, 2048, 256, 8, 32
NCORES = 8
HPC = (B * H) // NCORES  # heads per core = 2
HD = HPC * D  # 64
QT = S // 128  # 16 q (and k) tiles
NCH = S // 512  # 4 free-dim chunks of 512
BF16 = mybir.dt.bfloat16
F32 = mybir.dt.float32
EXPF = mybir.ActivationFunctionType.Exp
SIGF = mybir.ActivationFunctionType.Sigmoid
COPYF = mybir.ActivationFunctionType.Copy

_NC_CACHE = {}


def build_nc():
    nc = bacc.Bacc("TRN2", target_bir_lowering=False, debug=False, num_devices=NCORES)

    xq = nc.dram_tensor("xq", [C, S], BF16, kind="ExternalInput").ap()
    xkv = nc.dram_tensor("xkv", [C, S], BF16, kind="ExternalInput").ap()
    biasT = nc.dram_tensor("biasT", [HPC, S, S], BF16, kind="ExternalInput").ap()
    wq = nc.dram_tensor("wq", [C, HD], BF16, kind="ExternalInput").ap()
    wk = nc.dram_tensor("wk", [C, HD], BF16, kind="ExternalInput").ap()
    wv = nc.dram_tensor("wv", [C, HD], BF16, kind="ExternalInput").ap()
    wo = nc.dram_tensor("wo", [HD, C], BF16, kind="ExternalInput").ap()
    wg = nc.dram_tensor("wg", [C, C], BF16, kind="ExternalInput").ap()
    browg = nc.dram_tensor("browg", [1, C], BF16, kind="ExternalInput").ap()
    browo = nc.dram_tensor("browo", [1, C], BF16, kind="ExternalInput").ap()
    out = nc.dram_tensor("out", [S, C], F32, kind="ExternalOutput").ap()

    with tile.TileContext(nc, pool_alloc_mode="queue") as tc, ExitStack() as ctx:
        consts = ctx.enter_context(tc.tile_pool(name="consts", bufs=1))
        sb = ctx.enter_context(tc.tile_pool(name="sb", bufs=1))
        bias_pool = ctx.enter_context(tc.tile_pool(name="bias", bufs=3))
        exp_pool = ctx.enter_context(tc.tile_pool(name="exp", bufs=3))
        res_pool = ctx.enter_context(tc.tile_pool(name="res", bufs=6))
        ps_s = ctx.enter_context(tc.tile_pool(name="ps_s", bufs=3, space="PSUM"))
        ps_o = ctx.enter_context(tc.tile_pool(name="ps_o", bufs=1, space="PSUM"))

        # ---- constants ----
        id97 = consts.tile([97, 97], F32)
        make_identity(nc, id97[:])
        ones_r = consts.tile([1, 128], BF16)
        nc.vector.memset(ones_r[:], 1.0)
        ones_c = consts.tile([128, 1], BF16)
        nc.vector.memset(ones_c[:], 1.0)

        # ---- DMAs ordered by first consumer: gating needs wg + xq
        #      chunk 0; projections need wq/wk + the rest of x ----
        def load_w2(name, dram, m):
            t = consts.tile([128, 2 * m], BF16, tag=name, name=name + "_sb")
            nc.sync.dma_start(
                t[:].rearrange("p (j m) -> p j m", j=2),
                dram.rearrange("(j p) m -> p j m", p=128),
            )
            return t

        xq_sb = sb.tile([128, 2 * S], BF16)
        xkv_sb = sb.tile([128, 2 * S], BF16)

        def load_x(t_, dram, n):
            dst = t_[:].rearrange("p (j s) -> p j s", j=2)
            src = dram.rearrange("(j p) s -> p j s", p=128)
            nc.sync.dma_start(
                dst[:, :, ds(n * 512, 512)], src[:, :, ds(n * 512, 512)]
            )

        wg_sb = load_w2("wg", wg, C)
        browg_sb = consts.tile([1, C], BF16)
        nc.sync.dma_start(browg_sb[:], browg)
        browo_sb = consts.tile([1, C], BF16)
        nc.sync.dma_start(browo_sb[:], browo)
        load_x(xq_sb, xq, 0)
        wq_sb = load_w2("wq", wq, HD)
        wk_sb = load_w2("wk", wk, HD)
        wv_sb = load_w2("wv", wv, HD)
        load_x(xkv_sb, xkv, 0)
        for n in range(1, NCH):
            load_x(xq_sb, xq, n)
            load_x(xkv_sb, xkv, n)
        wo_sb = consts.tile([HD, C], BF16)
        nc.sync.dma_start(wo_sb[:], wo)

        # ---- projections: qT, kT [HD, S] bf16 (d on partitions),
        #      col-packed: qT in rows 0-63, kT in rows 64-127 ----
        qT = sb.tile([HD, S], BF16)
        kT = sb.tile([HD, S], BF16)
        for n in range(NCH):
            ps = ps_s.tile([128, 512], F32, tag="ps", name="ps_proj")
            for j in range(2):
                nc.tensor.matmul(
                    ps[ds(0, HD), :],
                    wq_sb[:, ds(j * HD, HD)],
                    xq_sb[:, ds(j * S + n * 512, 512)],
                    start=(j == 0), stop=(j == 1),
                    tile_position=(0, 0),
                )
                nc.tensor.matmul(
                    ps[ds(HD, HD), :],
                    wk_sb[:, ds(j * HD, HD)],
                    xkv_sb[:, ds(j * S + n * 512, 512)],
                    start=(j == 0), stop=(j == 1),
                    tile_position=(0, 64),
                )
            nc.vector.tensor_copy(qT[:, ds(n * 512, 512)], ps[ds(0, HD), :])
            nc.vector.tensor_copy(kT[:, ds(n * 512, 512)], ps[ds(HD, HD), :])

        # ---- gating: g = sigmoid(xq.T @ WgT + brow_g); first half in the
        #      prologue, second half in the tail (PE is idle there) ----
        g_all = sb.tile([128, QT * C], F32)

        def emit_gate(t):
            psg = ps_s.tile([128, C], F32, tag="ps", name="psg")
            for j in range(2):
                nc.tensor.matmul(
                    psg[:],
                    xq_sb[:, ds(j * S + t * 128, 128)],
                    wg_sb[:, ds(j * C, C)],
                    start=(j == 0), stop=False,
                )
            nc.tensor.matmul(psg[:], ones_r[:], browg_sb[:], start=False, stop=True)
            nc.scalar.activation(g_all[:, ds(t * C, C)], psg[:], SIGF)

        for t in range(QT // 2):
            emit_gate(t)

        # ---- b_out/4 broadcast to [128, C] ----
        bout_bc = consts.tile([128, C], F32)
        ps_b = ps_s.tile([128, C], F32, tag="ps")
        nc.tensor.matmul(ps_b[:], ones_r[:], browo_sb[:], start=True, stop=True)
        nc.vector.tensor_copy(bout_bc[:], ps_b[:])

        # ---- V natural layout: v_sb[:, kt*HD ...]; tiles are emitted
        #      lazily (2-kt lookahead) inside the score loop ----
        v_sb = sb.tile([128, QT * HPC * D], BF16)

        def emit_v(t):
            ps = ps_s.tile([128, HD], F32, tag="ps", name="ps_v")
            for j in range(2):
                nc.tensor.matmul(
                    ps[:],
                    xkv_sb[:, ds(j * S + t * 128, 128)],
                    wv_sb[:, ds(j * HD, HD)],
                    start=(j == 0),
                    stop=(j == 1),
                )
            nc.vector.tensor_copy(v_sb[:, ds(t * HD, HD)], ps[:])

        emit_v(0)
        emit_v(1)

        # ---- attention per head, software-pipelined (PV lags QK by 1 kt);
        #      head-0's epilogue half is spread through head-1's loop ----
        # folded layouts: oT_fold rows 32n = oT[0:32, 512n:512n+512]
        oT_sb = sb.tile([128, HPC * 512], BF16)
        oT_unf = sb.tile([HD, S], BF16)  # head i rows 32i, natural q columns
        den_sb = sb.tile([97, HPC * 512], F32)
        r97 = sb.tile([128, HPC * NCH * 97], F32)
        res0_all = sb.tile([128, QT * C], F32)
        oT_tiles = {}
        den_tiles = {}

        def emit_den_chain(i):
            """den psum -> den_sb -> transposed reciprocals in r97."""
            nc.vector.tensor_copy(den_sb[:, ds(i * 512, 512)], den_tiles[i][:])
            for c in range(NCH):
                trp = ps_s.tile([128, 97], F32, tag="ps", name="trp")
                nc.tensor.transpose(
                    trp[:], den_sb[:, ds(i * 512 + c * 128, 128)], id97[:]
                )
                nc.vector.reciprocal(
                    r97[:, ds((i * NCH + c) * 97, 97)][:, 0:97:32],
                    trp[:, 0:97:32],
                )

        def emit_oproj(i, t):
            """out-projection of head i for q-tile t, scaled by 1/den."""
            n, c = t // 4, t % 4
            ps = ps_s.tile([128, C], F32, tag="ps", name="ps_op")
            nc.tensor.matmul(
                ps[:],
                oT_unf[ds(i * D, D), ds(t * 128, 128)],
                wo_sb[ds(i * D, D), :],
                start=True, stop=True,
            )
            r_ap = r97[:, ds((i * NCH + c) * 97 + 32 * n, 1)]
            if i == 0:
                res0 = res0_all[:, ds(t * C, C)]
                nc.scalar.activation(res0, ps[:], COPYF, scale=r_ap)
                nc.vector.tensor_add(res0, res0, bout_bc[:])
            else:
                if t >= QT // 2:
                    emit_gate(t)
                tmp = res_pool.tile([128, C], F32, tag="tmp", name="tmp")
                nc.scalar.activation(tmp[:], ps[:], COPYF, scale=r_ap)
                res = res_pool.tile([128, C], F32, tag="res", name="res")
                nc.vector.tensor_add(res[:], res0_all[:, ds(t * C, C)], tmp[:])
                nc.vector.tensor_mul(res[:], res[:], g_all[:, ds(t * C, C)])
                nc.sync.dma_start(out[ds(t * 128, 128), :], res[:])

        prev = None
        for j in range(HPC * QT + 1):
            cur = None
            if j < HPC * QT:
                i, kt = j // QT, j % QT
                if kt == 0:
                    oT_tiles[i] = ps_o.tile(
                        [128, 512], F32, tag="ps_oT", name="oT_ps"
                    )
                    den_tiles[i] = ps_o.tile(
                        [97, 512], F32, tag="ps_den", name="den_ps"
                    )
                bt = bias_pool.tile([128, S], BF16)
                nc.sync.dma_start(bt[:], biasT[i, ds(kt * 128, 128), :])
                et = exp_pool.tile([128, S], BF16)
                pss = []
                for n in range(2):
                    ps = ps_s.tile([128, 1024], F32, tag="ps")
                    for m in range(2):
                        nc.tensor.matmul(
                            ps[:, ds(m * 512, 512)],
                            kT[ds(i * D, D), ds(kt * 128, 128)],
                            qT[ds(i * D, D), ds(n * 1024 + m * 512, 512)],
                            start=True, stop=True,
                        )
                    pss.append(ps)
                for n in range(2):
                    nc.scalar.activation(et[:, ds(n * 1024, 1024)], pss[n][:], EXPF)
                    nc.vector.tensor_mul(
                        et[:, ds(n * 1024, 1024)],
                        et[:, ds(n * 1024, 1024)],
                        bt[:, ds(n * 1024, 1024)],
                    )
                if i == 0 and kt + 2 < QT:
                    emit_v(kt + 2)
                cur = (i, kt, et)
            if prev is not None:
                pi, pkt, pet = prev
                st, sp = pkt == 0, pkt == QT - 1
                for n in range(NCH):
                    nc.tensor.matmul(
                        oT_tiles[pi][ds(32 * n, 32), :],
                        v_sb[:, ds((pkt * HPC + pi) * D, D)],
                        pet[:, ds(n * 512, 512)],
                        start=st, stop=sp,
                        tile_position=(0, 32 * n),
                    )
                for n in range(NCH):
                    nc.tensor.matmul(
                        den_tiles[pi][ds(32 * n, 1), :],
                        ones_c[:],
                        pet[:, ds(n * 512, 512)],
                        start=st, stop=sp,
                        tile_position=(0, 32 * n),
                    )
                if sp:
                    nc.vector.tensor_copy(
                        oT_sb[:, ds(pi * 512, 512)], oT_tiles[pi][:]
                    )
                    # unfold: oT_unf[32i+p, 512n+j] = oT_sb[32n+p, 512i+j]
                    for n in range(NCH):
                        nc.scalar.dma_start(
                            oT_unf[ds(pi * D, D), ds(n * 512, 512)],
                            oT_sb[ds(32 * n, 32), ds(pi * 512, 512)],
                        )
            # spread head-0 epilogue through head-1's score loop
            if j == QT:
                emit_den_chain(0)
            elif QT + 1 <= j <= 2 * QT:
                emit_oproj(0, j - QT - 1)
            prev = cur

        # ---- tail: head-1 epilogue ----
        emit_den_chain(1)
        for t in range(QT):
            emit_oproj(1, t)

    nc.compile()
    return nc


def _shard_inputs(q_x, kv_x, attn_bias, Wq, Wk, Wv, Wout, b_out, Wg, b_g, gating_bias):
    bf = ml_dtypes.bfloat16
    in_maps = []
    scale = np.float32(D) ** np.float32(-0.5)
    for core in range(NCORES):
        b, hp = core // 4, core % 4
        hsl = slice(hp * HD, (hp + 1) * HD)
        in_maps.append(
            {
                "xq": np.ascontiguousarray(q_x[b].T).astype(bf),
                "xkv": np.ascontiguousarray(kv_x[b].T).astype(bf),
                "biasT": np.exp(
                    np.ascontiguousarray(
                        attn_bias[b, 2 * hp : 2 * hp + 2].transpose(0, 2, 1)
                    )
                ).astype(bf),
                "wq": np.ascontiguousarray((Wq[hsl] * scale).T).astype(bf),
                "wk": np.ascontiguousarray(Wk[hsl].T).astype(bf),
                "wv": np.ascontiguousarray(Wv[hsl].T).astype(bf),
                "wo": np.ascontiguousarray(Wout[:, hsl].T).astype(bf),
                "wg": np.ascontiguousarray(Wg.T).astype(bf),
                "browg": (b_g + gating_bias).reshape(1, C).astype(bf),
                "browo": (b_out / 4.0).reshape(1, C).astype(bf),
            }
        )
    return in_maps


def run(inputs, trace=False, **kw):
    if "nc" not in _NC_CACHE:
        _NC_CACHE["nc"] = build_nc()
    nc = _NC_CACHE["nc"]
    inputs = {k: np.asarray(v, dtype=np.float32) for k, v in inputs.items()}
    in_maps = _shard_inputs(**inputs)
    r = run_bass_kernel_spmd(nc, in_maps, core_ids=list(range(NCORES)), trace=trace, **kw)
    outs = np.stack([np.asarray(m["out"], np.float32) for m in r.results])
    full = outs.reshape(B, 4, S, C).sum(axis=1)
    return full, r


def kernel(**inputs) -> np.ndarray:
    full, _ = run(inputs, trace=False)
    return full


if __name__ == "__main__":
    print("building...")
    build_nc()
    print("ok")
